# revision 18
# baseline (speedup 1.0000x reference)
"""Trainium2 Bass kernel for a ResNet BasicBlock (dense CNN, sync-BN).

Reference computation (training-mode BN, batch stats over (N,H,W)):
    h = conv3x3(x, W1) * mask1            # structured channel pruning
    h = relu(bn(h, gamma1, beta1))
    h = conv3x3(h, W2) * mask2
    h = bn(h, gamma2, beta2)
    out = relu(h + x)                      # identity shortcut

Shapes: x [32, 256, 56, 56] f32, W [256, 256, 3, 3] f32.

Strategy: data-parallel over batch N across 8 NeuronCores (4 images per
core), weights replicated.  BN batch statistics are synchronized with a
tiny (1 KB/half) AllReduce of per-channel (sum, sum-of-squares) pairs.

Per-core layout:
  - Channels are split into two 128-partition halves (C=256 = 2*128).
  - Conv inputs live in SBUF as zero-padded 58x58 bf16 planes (row
    stride 58), so each of the 9 taps of the 3x3 conv is a plain offset
    shift: one matmul per (tap, ci-half) accumulating into PSUM.
  - Masks are folded into the weights on the host (zero rows).

Scheduling (the whole point of this file):
  - Sync-BN makes every exchange a global barrier, and NEFF dispatch
    staggers core starts by ~60-70us, so the PE would idle for the
    cross-core skew at each BN boundary.  conv2 is therefore split in
    two passes: pass A accumulates only the ci-half-0 taps (needs only
    BN1 coefficients for half 0, whose stats complete a whole conv-half
    earlier) into a partial h2 (bf16); pass B reloads the partial into
    PSUM with an identity matmul and adds the ci-half-1 taps.  The PE
    rolls conv1 -> pass A -> pass B with no stats wall.
  - BN1 is applied in-place on the idle DVE stream right after each
    recv/affine, never touching the ACT stream (which feeds PSUM
    rotation; a blocked ACT op stalls the PE).
  - The chip is power/DVFS-limited: aux engine work is kept to short
    bursts (measured: heavy DVE+Pool during conv drops the PE from
    2.35GHz to 1.95GHz and slows DVE ops 5x).
  - tail half 0 runs on DVE overlapped under pass-B half 1; tail half 1
    splits DVE/Pool post-conv with ACT relus pinned after the last conv
    ACT op (the list scheduler would otherwise hoist stats-gated ops
    into the conv copy stream: head-of-line deadlock on PSUM).
  - gpsimd sem_clears wait on the cross-core entry barrier; nothing
    queues behind them on the Pool engine.
"""

import numpy as np
import ml_dtypes

# ---- problem constants (hardcoded; kernel.py must be self-contained) ----
N_TOT, C, H, W = 32, 256, 56, 56
N_CORES = 8
NL = N_TOT // N_CORES          # images per core
PW = H + 2                     # padded row stride (58)
PLANE = PW * PW + 4            # padded plane floats + 4 spare for tap overreads
STRIP0 = PW + 1                # first interior output position (59)
CHUNK = 8 * PW                 # 464: 8 output rows per chunk
NCHUNK = 7                     # 7 chunks * 8 rows = 56 rows
HW = H * W                     # 3136
HALF_ROWS = 28                 # row granularity for x/out streaming DMAs
HALF_ELEMS = HALF_ROWS * W     # 1568
COUNT = N_TOT * HW             # sync-BN element count per channel
EPS = 1e-5

_BF16 = ml_dtypes.bfloat16

_cache = {}


def _pack_weights(W1, W2, mask1, mask2):
    """-> [128, 72*128] bf16: [i, (conv,ky,kx,ci,co), o] with masks folded."""
    Wm = np.stack([W1 * mask1[:, None, None, None],
                   W2 * mask2[:, None, None, None]]).astype(np.float32)
    # [conv, O, I, 3, 3] -> [conv, co, o, ci, i, ky, kx]
    Wr = Wm.reshape(2, 2, 128, 2, 128, 3, 3)
    # -> [conv, ky, kx, ci, co, i, o]
    A = Wr.transpose(0, 5, 6, 3, 1, 4, 2)
    # -> [i, t, o] -> [128, 72*128]
    B = np.ascontiguousarray(A.transpose(5, 0, 1, 2, 3, 4, 6)).reshape(128, 72 * 128)
    return B.astype(_BF16)


def _t_index(ky, kx, ci, co):
    # tap index within one conv's 36-block
    return co + 2 * (ci + 2 * (kx + 3 * ky))


def _pack_aff(gamma1, beta1, gamma2, beta2):
    cols = [gamma1, beta1, gamma2, beta2]
    out = np.empty((128, 8), np.float32)
    for k, v in enumerate(cols):
        v = np.asarray(v, np.float32).reshape(2, 128)
        out[:, 2 * k] = v[0]
        out[:, 2 * k + 1] = v[1]
    return out


def _build():
    import concourse.bass as bass_mod
    import concourse.bacc as bacc
    import concourse.mybir as mybir
    import concourse.tile as tile

    f32 = mybir.dt.float32
    bf16 = mybir.dt.bfloat16
    AX = mybir.AxisListType
    ALU = mybir.AluOpType
    AF = mybir.ActivationFunctionType

    nc = bacc.Bacc("TRN2", target_bir_lowering=False, debug=False,
                   num_devices=N_CORES, num_swdge_queues=4)

    x_d = nc.dram_tensor("x", [NL, C, H, W], bf16, kind="ExternalInput")
    wt_d = nc.dram_tensor("wt", [128, 72 * 128], bf16, kind="ExternalInput")
    aff_d = nc.dram_tensor("aff", [128, 8], f32, kind="ExternalInput")
    eye_d = nc.dram_tensor("eye", [128, 128], bf16, kind="ExternalInput")
    out_d = nc.dram_tensor("out", [NL, C, H, W], f32, kind="ExternalOutput")

    groups = [list(range(N_CORES))]

    def interior(tile_ap, base, nrows):
        """[128, nrows, 56] strided view (row stride PW) starting at `base`."""
        v = tile_ap[:, base:base + nrows * PW].rearrange(
            "p (r c) -> p r c", c=PW)
        return v[:, :, 0:W]

    with tile.TileContext(nc) as tc:
        import contextlib
        with contextlib.ExitStack() as ctx:
            const = ctx.enter_context(tc.tile_pool(name="const", bufs=1))
            psum = ctx.enter_context(tc.tile_pool(name="psum", bufs=8, space="PSUM"))
            xst = ctx.enter_context(tc.tile_pool(name="xst", bufs=2))
            otp = ctx.enter_context(tc.tile_pool(name="otp", bufs=2))
            sqp = ctx.enter_context(tc.tile_pool(name="sqp", bufs=2))

            # persistent per-image planes
            x_pad = [[const.tile([128, PLANE], bf16, tag=f"xp{j}_{n}", name=f"xp{j}_{n}")
                      for n in range(NL)] for j in range(2)]
            h1_pad = [[const.tile([128, PLANE], bf16, tag=f"h1{j}_{n}", name=f"h1{j}_{n}")
                       for n in range(NL)] for j in range(2)]
            h2 = [[const.tile([128, HW], bf16, tag=f"h2{j}_{n}", name=f"h2{j}_{n}")
                   for n in range(NL)] for j in range(2)]

            wt_sb = [const.tile([128, 36 * 128], bf16, tag=f"wt{c}", name=f"wt{c}")
                     for c in range(2)]
            aff_sb = const.tile([128, 8], f32, tag="aff", name="aff")
            eye_sb = const.tile([128, 128], bf16, tag="eye", name="eye")

            rsem = [nc.alloc_semaphore(f"rst{i}") for i in range(4)]
            lsem = nc.alloc_semaphore("lst")
            _gp_prev = [None]
            deferred_waits = []

            def gp_order(bi):
                if _gp_prev[0] is not None:
                    bass_mod._add_dep_helper(bi.ins, _gp_prev[0].ins,
                                             sync=False,
                                             reason="stats-exchange order")
                _gp_prev[0] = bi
                return bi

            nc._bir_kernel_barrier_sem_replica_groups.extend(
                set(g) for g in groups)

            def defer_wait(bi, sem, val):
                bi._wait_ge(sem, 0)
                deferred_waits.append((bi, sem, val))
                return bi

            for i, s in enumerate(rsem + [lsem]):
                cl = gp_order(nc.gpsimd.sem_clear(s))
                if i == 0:
                    defer_wait(cl, nc._bir_kernel_barrier_sem,
                               nc.bir_kernel_barrier_sem_inc)

            # stats accumulators: one column per (image, chunk)
            acc = {(b, s, j): const.tile([128, NL * NCHUNK], f32,
                                         tag=f"acc{b}{s}{j}", name=f"acc{b}{s}{j}")
                   for b in (1, 2) for s in ("s", "q") for j in range(2)}

            rv = [const.tile([128, 16], f32, tag=f"rv{ex}", name=f"rv{ex}")
                  for ex in range(4)]
            packed = [const.tile([128, 2], f32, tag=f"pk{ex}", name=f"pk{ex}")
                      for ex in range(4)]

            def pad_memset(t):
                nc.vector.memset(t[:, 0:STRIP0], 0.0)
                pairs = t[:, 2 * PW - 1:2 * PW - 1 + 56 * PW].rearrange(
                    "p (r c) -> p r c", c=PW)[:, :, 0:2]
                nc.vector.memset(pairs, 0.0)
                nc.vector.memset(t[:, STRIP0 + 56 * PW:PLANE], 0.0)

            # ---- head: image-0 x DMAs issue first (Sync engine issues one
            # dma_start per ~0.75us, so order = priority), weights after. ----
            def x_load(n, j):
                for rh in range(2):
                    r0 = rh * HALF_ROWS
                    xs = xst.tile([128, HALF_ELEMS], bf16, tag="xs", name="xs")
                    nc.sync.dma_start(
                        xs[:], x_d[n, j * 128:(j + 1) * 128, r0:r0 + HALF_ROWS, :])
                    dst = interior(x_pad[j][n], (r0 + 1) * PW + 1, HALF_ROWS)
                    nc.vector.tensor_copy(
                        dst, xs[:, :].rearrange("p (r c) -> p r c", c=W))

            x_load(0, 0)
            x_load(0, 1)
            nc.sync.dma_start(wt_sb[0][:], wt_d[:, 0:36 * 128])
            pad_memset(x_pad[0][0])
            pad_memset(x_pad[1][0])
            for n in range(1, NL):
                for j in range(2):
                    x_load(n, j)
                pad_memset(x_pad[0][n])
                pad_memset(x_pad[1][n])
            nc.sync.dma_start(wt_sb[1][:], wt_d[:, 36 * 128:72 * 128])
            nc.sync.dma_start(aff_sb[:], aff_d[:])
            nc.sync.dma_start(eye_sb[:], eye_d[:])

            # ---- stats-exchange pieces ----
            def stats_send(bn_i, j, ex):
                nc.vector.tensor_reduce(
                    packed[ex][:, 0:1], acc[(bn_i, "s", j)][:], axis=AX.X, op=ALU.add)
                nc.vector.tensor_reduce(
                    packed[ex][:, 1:2], acc[(bn_i, "q", j)][:], axis=AX.X, op=ALU.add)
                cp = nc.vector.tensor_copy(rv[ex][:, 0:2], packed[ex][:])
                for d in range(1, 8):
                    q = (d - 1) % 4
                    rd = [None] * 8
                    rd[d] = (0, d)
                    gp_order(nc.gpsimd.remote_dma_broadcast(
                        rv[ex][:, 2 * d:2 * d + 2], packed[ex][:],
                        remote_sem=rsem[ex], local_sem=lsem, rdests=rd,
                        queue_num=q))
                for q in range(4):
                    gp_order(nc.gpsimd.trigger_dma(count=None, queue_num=q))
                return cp

            def stats_recv(bn_i, j, ex, after_bis):
                sfx = f"{bn_i}{j}"
                gl = const.tile([128, 2], f32, tag=f"gl{sfx}", name=f"gl{sfx}")
                red = nc.vector.tensor_reduce(
                    gl[:], rv[ex][:, 0:16].rearrange("p (s c) -> p c s", c=2),
                    axis=AX.X, op=ALU.add)
                defer_wait(red, rsem[ex], 14)
                for ab in after_bis:
                    bass_mod._add_dep_helper(red.ins, ab.ins, sync=True,
                                             reason="recv ordering")
                return gl

            def bn_affine_finish_fast(bn_i, j, gl, g_col, b_col, act_pin):
                """ACT-sqrt + DVE-reciprocal affine: ~1.5us instead of the
                ~3.8us DVE bit-trick chain.  Only legal post-conv (the sqrt
                sits in the ACT stream; act_pin keeps the scheduler from
                hoisting it into the conv copy stream)."""
                sfx = f"f{bn_i}{j}"
                mean = const.tile([128, 1], f32, tag=f"mean{sfx}", name=f"mean{sfx}")
                nc.vector.tensor_scalar_mul(mean[:], gl[:, 0:1], 1.0 / COUNT)
                var = const.tile([128, 1], f32, tag=f"var{sfx}", name=f"var{sfx}")
                nc.vector.tensor_tensor(var[:], mean[:], mean[:], ALU.mult)
                nc.vector.scalar_tensor_tensor(
                    var[:], gl[:, 1:2], 1.0 / COUNT, var[:],
                    ALU.mult, ALU.subtract)
                nc.vector.tensor_scalar_add(var[:], var[:], EPS)
                sd = const.tile([128, 1], f32, tag=f"sd{sfx}", name=f"sd{sfx}")
                sq = nc.scalar.sqrt(sd[:], var[:])
                bass_mod._add_dep_helper(sq.ins, act_pin.ins, sync=True,
                                         reason="affine sqrt after conv ACT")
                y = const.tile([128, 1], f32, tag=f"y{sfx}", name=f"y{sfx}")
                nc.vector.reciprocal(y[:], sd[:])
                sc = const.tile([128, 1], f32, tag=f"sc{sfx}", name=f"sc{sfx}")
                nc.vector.tensor_tensor(sc[:], aff_sb[:, g_col + j:g_col + j + 1],
                                        y[:], ALU.mult)
                bi = const.tile([128, 1], f32, tag=f"bi{sfx}", name=f"bi{sfx}")
                nc.vector.tensor_tensor(bi[:], mean[:], sc[:], ALU.mult)
                nc.vector.tensor_tensor(bi[:], aff_sb[:, b_col + j:b_col + j + 1],
                                        bi[:], ALU.subtract)
                return sc, bi

            def bn_affine_finish(bn_i, j, gl, g_col, b_col):
                sfx = f"{bn_i}{j}"
                mean = const.tile([128, 1], f32, tag=f"mean{sfx}", name=f"mean{sfx}")
                nc.vector.tensor_scalar_mul(mean[:], gl[:, 0:1], 1.0 / COUNT)
                var = const.tile([128, 1], f32, tag=f"var{sfx}", name=f"var{sfx}")
                nc.vector.tensor_tensor(var[:], mean[:], mean[:], ALU.mult)
                nc.vector.scalar_tensor_tensor(
                    var[:], gl[:, 1:2], 1.0 / COUNT, var[:],
                    ALU.mult, ALU.subtract)
                nc.vector.tensor_scalar_add(var[:], var[:], EPS)
                y = const.tile([128, 1], f32, tag=f"y{sfx}", name=f"y{sfx}")
                vh = const.tile([128, 1], f32, tag=f"vh{sfx}", name=f"vh{sfx}")
                tmp = const.tile([128, 1], f32, tag=f"tm{sfx}", name=f"tm{sfx}")
                iv = var[:].bitcast(mybir.dt.int32)
                yi = y[:].bitcast(mybir.dt.int32)
                nc.vector.tensor_scalar(yi, iv, 1, None, ALU.arith_shift_right)
                nc.vector.tensor_scalar(yi, yi, -1, None, ALU.bitwise_xor)
                nc.vector.tensor_scalar(yi, yi, 0x5f3759df + 1, None, ALU.add)
                nc.vector.tensor_scalar_mul(vh[:], var[:], 0.5)
                for _ in range(2):
                    nc.vector.tensor_tensor(tmp[:], y[:], y[:], ALU.mult)
                    nc.vector.tensor_tensor(tmp[:], tmp[:], vh[:], ALU.mult)
                    nc.vector.tensor_scalar(tmp[:], tmp[:], -1.0, 1.5,
                                            ALU.mult, ALU.add)
                    nc.vector.tensor_tensor(y[:], y[:], tmp[:], ALU.mult)
                sc = const.tile([128, 1], f32, tag=f"sc{sfx}", name=f"sc{sfx}")
                nc.vector.tensor_tensor(sc[:], aff_sb[:, g_col + j:g_col + j + 1],
                                        y[:], ALU.mult)
                bi = const.tile([128, 1], f32, tag=f"bi{sfx}", name=f"bi{sfx}")
                nc.vector.tensor_tensor(bi[:], mean[:], sc[:], ALU.mult)
                nc.vector.tensor_tensor(bi[:], aff_sb[:, b_col + j:b_col + j + 1],
                                        bi[:], ALU.subtract)
                return sc, bi

            def apply_bn1_dve(j, s, b):
                """relu(s*h1+b) in place on DVE (never blocks the ACT/PSUM
                pipeline; runs in the recv/affine shadow)."""
                for n in range(NL):
                    v = interior(h1_pad[j][n], STRIP0, H)
                    nc.vector.tensor_scalar(v, v, s[:], b[:], ALU.mult, ALU.add)
                    nc.vector.tensor_scalar(v, v, 0.0, None, ALU.max)

            # ---- conv pass emitters ----
            def conv1(j):
                last = None
                for n in range(NL):
                    for k in range(NCHUNK):
                        pt = psum.tile([128, 8 * W], f32, tag="ps", name="ps")
                        idx = 0
                        for ci in range(2):
                            for ky in range(3):
                                for kx in range(3):
                                    t = _t_index(ky, kx, ci, j)
                                    dq = (ky - 1) * PW + (kx - 1)
                                    off = STRIP0 + CHUNK * k + dq
                                    rhs = x_pad[ci][n][
                                        :, off:off + CHUNK].rearrange(
                                        "p (r c) -> p r c", c=PW)[:, :, 0:W]
                                    nc.tensor.matmul(
                                        pt[:], wt_sb[0][:, t * 128:(t + 1) * 128],
                                        rhs, start=(idx == 0), stop=(idx == 17))
                                    idx += 1
                        src_int = pt[:, 0:8 * W].rearrange("p (r c) -> p r c", c=W)
                        dst_int = interior(h1_pad[j][n], (1 + 8 * k) * PW + 1, 8)
                        col = n * NCHUNK + k
                        nc.scalar.activation(
                            dst_int, src_int, AF.Copy,
                            accum_out=acc[(1, "s", j)][:, col:col + 1])
                        sq = sqp.tile([128, 8 * W], f32, tag="sq", name="sq")
                        last = nc.scalar.activation(
                            sq[:, :].rearrange("p (r c) -> p r c", c=W),
                            dst_int, AF.Square,
                            accum_out=acc[(1, "q", j)][:, col:col + 1])
                return last

            def conv2_passA(j):
                """ci-half-0 taps only -> h2[j] partial (bf16, no stats)."""
                for n in range(NL):
                    for k in range(NCHUNK):
                        pt = psum.tile([128, 8 * W], f32, tag="ps", name="ps")
                        for ti, (ky, kx) in enumerate(
                                (ky, kx) for ky in range(3) for kx in range(3)):
                            t = 36 + _t_index(ky, kx, 0, j)
                            dq = (ky - 1) * PW + (kx - 1)
                            off = STRIP0 + CHUNK * k + dq
                            rhs = h1_pad[0][n][:, off:off + CHUNK].rearrange(
                                "p (r c) -> p r c", c=PW)[:, :, 0:W]
                            nc.tensor.matmul(
                                pt[:], wt_sb[1][:, (t - 36) * 128:(t - 35) * 128],
                                rhs, start=(ti == 0), stop=(ti == 8))
                        dst = h2[j][n][:, 8 * k * W:(8 * k + 8) * W].rearrange(
                            "p (r c) -> p r c", c=W)
                        nc.scalar.activation(
                            dst, pt[:, 0:8 * W].rearrange("p (r c) -> p r c", c=W),
                            AF.Copy)

            def conv2_passB(j):
                """identity-reload of the partial + ci-half-1 taps + stats."""
                last = None
                for n in range(NL):
                    for k in range(NCHUNK):
                        pt = psum.tile([128, 8 * W], f32, tag="ps", name="ps")
                        part = h2[j][n][:, 8 * k * W:(8 * k + 8) * W].rearrange(
                            "p (r c) -> p r c", c=W)
                        nc.tensor.matmul(pt[:], eye_sb[:], part,
                                         start=True, stop=False)
                        for ti, (ky, kx) in enumerate(
                                (ky, kx) for ky in range(3) for kx in range(3)):
                            t = 36 + _t_index(ky, kx, 1, j)
                            dq = (ky - 1) * PW + (kx - 1)
                            off = STRIP0 + CHUNK * k + dq
                            rhs = h1_pad[1][n][:, off:off + CHUNK].rearrange(
                                "p (r c) -> p r c", c=PW)[:, :, 0:W]
                            nc.tensor.matmul(
                                pt[:], wt_sb[1][:, (t - 36) * 128:(t - 35) * 128],
                                rhs, start=False, stop=(ti == 8))
                        src_int = pt[:, 0:8 * W].rearrange("p (r c) -> p r c", c=W)
                        dst = h2[j][n][:, 8 * k * W:(8 * k + 8) * W].rearrange(
                            "p (r c) -> p r c", c=W)
                        col = n * NCHUNK + k
                        nc.scalar.activation(
                            dst, src_int, AF.Copy,
                            accum_out=acc[(2, "s", j)][:, col:col + 1])
                        sq = sqp.tile([128, 8 * W], f32, tag="sq", name="sq")
                        last = nc.scalar.activation(
                            sq[:, :].rearrange("p (r c) -> p r c", c=W), dst,
                            AF.Square,
                            accum_out=acc[(2, "q", j)][:, col:col + 1])
                return last

            # ---- tail: out = relu(s2*h2 + b2 + x), stream to DRAM. ----
            def tail(j, s2, b2, mode, pin_to):
                last_dve = None
                for m, (n, rh) in enumerate((n, rh) for n in range(NL)
                                            for rh in range(2)):
                    r0 = rh * HALF_ROWS
                    xv = interior(x_pad[j][n], (r0 + 1) * PW + 1, HALF_ROWS)
                    h2v = h2[j][n][:, r0 * W:r0 * W + HALF_ELEMS].rearrange(
                        "p (r c) -> p r c", c=W)
                    pool = otp if m % 2 == 0 else xst
                    ot = pool.tile([128, HALF_ELEMS], f32,
                                   tag="ot" if m % 2 == 0 else "xs", name="ot")
                    otv = ot[:, :].rearrange("p (r c) -> p r c", c=W)
                    if mode == "dve":
                        stt = nc.vector.scalar_tensor_tensor(
                            otv, h2v, s2[:], xv, ALU.mult, ALU.add)
                        if m == 0 and pin_to is not None:
                            bass_mod._add_dep_helper(stt.ins, pin_to.ins,
                                                     sync=True,
                                                     reason="tail order")
                        last_dve = nc.vector.tensor_scalar(
                            ot[:], ot[:], b2[:], 0.0, ALU.add, ALU.max)
                    else:
                        # bf16 in-place add-residual on the DVE (2 elem/cycle:
                        # ~0.95us vs 1.85us f32); the ACT relu converts
                        # bf16 -> f32 on the way to the output tile
                        stt = nc.vector.scalar_tensor_tensor(
                            h2v, h2v, s2[:], xv, ALU.mult, ALU.add)
                        if m == 0 and pin_to is not None:
                            bass_mod._add_dep_helper(stt.ins, pin_to.ins,
                                                     sync=True,
                                                     reason="tail order")
                        last_dve = stt
                        rl = nc.scalar.activation(otv, h2v, AF.Relu,
                                                  bias=b2[:], scale=1.0)
                        if pin_to is not None:
                            bass_mod._add_dep_helper(rl.ins, pin_to.ins,
                                                     sync=True,
                                                     reason="tail relu pin")
                    nc.sync.dma_start(
                        out_d[n, j * 128:(j + 1) * 128, r0:r0 + HALF_ROWS, :],
                        ot[:])
                return last_dve

            # ================= phase schedule =================
            conv1(0)                                         # conv1 half 0
            cp10 = stats_send(1, 0, 0)
            # h1 pad zeroing on DVE, emitted BEFORE the blocking recv so
            # conv1-half-1's ACT copies can never queue behind it
            for jj in range(2):
                for n in range(NL):
                    pad_memset(h1_pad[jj][n])
            gl10 = stats_recv(1, 0, 0, [cp10])               # blocks idle DVE
            s1_0, b1_0 = bn_affine_finish(1, 0, gl10, 0, 2)
            apply_bn1_dve(0, s1_0, b1_0)                     # in recv shadow
            c1_last = conv1(1)                               # conv1 half 1
            cp11 = stats_send(1, 1, 1)
            gl11 = stats_recv(1, 1, 1, [cp11])
            s1_1, b1_1 = bn_affine_finish(1, 1, gl11, 0, 2)
            apply_bn1_dve(1, s1_1, b1_1)
            conv2_passA(0)                                   # needs apply j0 only
            conv2_passA(1)
            c2a_last = conv2_passB(0)                        # needs apply j1
            cp20 = stats_send(2, 0, 2)
            gl20 = stats_recv(2, 0, 2, [cp20])
            s2_0, b2_0 = bn_affine_finish(2, 0, gl20, 4, 6)
            # tail half 0: DVE-only, overlaps pass-B half 1
            t0_last = tail(0, s2_0, b2_0, mode="dve", pin_to=cp20)
            c2_last = conv2_passB(1)                         # conv2 half 1
            cp21 = stats_send(2, 1, 3)
            gl21 = stats_recv(2, 1, 3, [cp21, t0_last])
            s2_1, b2_1 = bn_affine_finish_fast(2, 1, gl21, 4, 6, c2_last)
            tail(1, s2_1, b2_1, mode="mixed", pin_to=c2_last)

    for bi, sem, val in deferred_waits:
        patched = False
        for w in bi.ins.sync_info.on_wait:
            if w.id == sem.num and w.wait_value == 0:
                w.wait_value = val
                patched = True
                break
        assert patched, f"deferred wait not found on {bi.ins.name}"

    nc.compile()
    return nc


def kernel(x, W1, W2, gamma1, beta1, gamma2, beta2, mask1, mask2,
           _trace=False, _trace_kwargs=None):
    from concourse.bass_utils import run_bass_kernel_spmd

    if "nc" not in _cache:
        _cache["nc"] = _build()
    nc = _cache["nc"]

    wt = _pack_weights(np.asarray(W1, np.float32), np.asarray(W2, np.float32),
                       np.asarray(mask1, np.float32), np.asarray(mask2, np.float32))
    aff = _pack_aff(gamma1, beta1, gamma2, beta2)
    eye = np.eye(128, dtype=_BF16)
    x = np.ascontiguousarray(np.asarray(x, np.float32).astype(_BF16))

    in_maps = [{"x": x[i * NL:(i + 1) * NL], "wt": wt, "aff": aff, "eye": eye}
               for i in range(N_CORES)]
    kw = {}
    if _trace:
        kw = dict(trace=True, **(_trace_kwargs or {}))
    res = run_bass_kernel_spmd(nc, in_maps, core_ids=list(range(N_CORES)), **kw)
    out = np.concatenate([res.results[i]["out"] for i in range(N_CORES)], axis=0)
    _cache["last_results"] = res
    return out


# revision 19
# speedup vs baseline: 1.0064x; 1.0064x over previous
"""Trainium2 Bass kernel for a ResNet BasicBlock (dense CNN, sync-BN).

Reference computation (training-mode BN, batch stats over (N,H,W)):
    h = conv3x3(x, W1) * mask1            # structured channel pruning
    h = relu(bn(h, gamma1, beta1))
    h = conv3x3(h, W2) * mask2
    h = bn(h, gamma2, beta2)
    out = relu(h + x)                      # identity shortcut

Shapes: x [32, 256, 56, 56] f32, W [256, 256, 3, 3] f32.

Strategy: data-parallel over batch N across 8 NeuronCores (4 images per
core), weights replicated.  BN batch statistics are synchronized with a
tiny (1 KB/half) AllReduce of per-channel (sum, sum-of-squares) pairs.

Per-core layout:
  - Channels are split into two 128-partition halves (C=256 = 2*128).
  - Conv inputs live in SBUF as zero-padded 58x58 bf16 planes (row
    stride 58), so each of the 9 taps of the 3x3 conv is a plain offset
    shift: one matmul per (tap, ci-half) accumulating into PSUM.
  - Masks are folded into the weights on the host (zero rows).

Scheduling (the whole point of this file):
  - Sync-BN makes every exchange a global barrier, and NEFF dispatch
    staggers core starts by ~60-70us, so the PE would idle for the
    cross-core skew at each BN boundary.  conv2 is therefore split in
    two passes: pass A accumulates only the ci-half-0 taps (needs only
    BN1 coefficients for half 0, whose stats complete a whole conv-half
    earlier) into a partial h2 (bf16); pass B reloads the partial into
    PSUM with an identity matmul and adds the ci-half-1 taps.  The PE
    rolls conv1 -> pass A -> pass B with no stats wall.
  - BN1 is applied in-place on the idle DVE stream right after each
    recv/affine, never touching the ACT stream (which feeds PSUM
    rotation; a blocked ACT op stalls the PE).
  - The chip is power/DVFS-limited: aux engine work is kept to short
    bursts (measured: heavy DVE+Pool during conv drops the PE from
    2.35GHz to 1.95GHz and slows DVE ops 5x).
  - tail half 0 runs on DVE overlapped under pass-B half 1; tail half 1
    splits DVE/Pool post-conv with ACT relus pinned after the last conv
    ACT op (the list scheduler would otherwise hoist stats-gated ops
    into the conv copy stream: head-of-line deadlock on PSUM).
  - gpsimd sem_clears wait on the cross-core entry barrier; nothing
    queues behind them on the Pool engine.
"""

import numpy as np
import ml_dtypes

# ---- problem constants (hardcoded; kernel.py must be self-contained) ----
N_TOT, C, H, W = 32, 256, 56, 56
N_CORES = 8
NL = N_TOT // N_CORES          # images per core
PW = H + 2                     # padded row stride (58)
PLANE = PW * PW + 4            # padded plane floats + 4 spare for tap overreads
STRIP0 = PW + 1                # first interior output position (59)
CHUNK = 8 * PW                 # 464: 8 output rows per chunk
NCHUNK = 7                     # 7 chunks * 8 rows = 56 rows
HW = H * W                     # 3136
HALF_ROWS = 28                 # row granularity for x/out streaming DMAs
HALF_ELEMS = HALF_ROWS * W     # 1568
COUNT = N_TOT * HW             # sync-BN element count per channel
EPS = 1e-5

_BF16 = ml_dtypes.bfloat16

_cache = {}


def _pack_weights(W1, W2, mask1, mask2):
    """-> [128, 72*128] bf16: [i, (conv,ky,kx,ci,co), o] with masks folded."""
    Wm = np.stack([W1 * mask1[:, None, None, None],
                   W2 * mask2[:, None, None, None]]).astype(np.float32)
    # [conv, O, I, 3, 3] -> [conv, co, o, ci, i, ky, kx]
    Wr = Wm.reshape(2, 2, 128, 2, 128, 3, 3)
    # -> [conv, co, ky, kx, ci, i, o]  (co-major: each conv-half's 18 tap
    # blocks are contiguous, so conv1-half-0 only waits on a 2.3KB DMA)
    A = Wr.transpose(0, 1, 5, 6, 3, 4, 2)
    # -> [i, t, o] -> [128, 72*128]
    B = np.ascontiguousarray(A.transpose(5, 0, 1, 2, 3, 4, 6)).reshape(128, 72 * 128)
    return B.astype(_BF16)


def _t_index(ky, kx, ci, co):
    # tap index within one conv's 36-block (co-major layout)
    return co * 18 + 2 * (3 * ky + kx) + ci


def _pack_aff(gamma1, beta1, gamma2, beta2):
    cols = [gamma1, beta1, gamma2, beta2]
    out = np.empty((128, 8), np.float32)
    for k, v in enumerate(cols):
        v = np.asarray(v, np.float32).reshape(2, 128)
        out[:, 2 * k] = v[0]
        out[:, 2 * k + 1] = v[1]
    return out


def _build():
    import concourse.bass as bass_mod
    import concourse.bacc as bacc
    import concourse.mybir as mybir
    import concourse.tile as tile

    f32 = mybir.dt.float32
    bf16 = mybir.dt.bfloat16
    AX = mybir.AxisListType
    ALU = mybir.AluOpType
    AF = mybir.ActivationFunctionType

    nc = bacc.Bacc("TRN2", target_bir_lowering=False, debug=False,
                   num_devices=N_CORES, num_swdge_queues=4)

    x_d = nc.dram_tensor("x", [NL, C, H, W], bf16, kind="ExternalInput")
    wt_d = nc.dram_tensor("wt", [128, 72 * 128], bf16, kind="ExternalInput")
    aff_d = nc.dram_tensor("aff", [128, 8], f32, kind="ExternalInput")
    eye_d = nc.dram_tensor("eye", [128, 128], bf16, kind="ExternalInput")
    out_d = nc.dram_tensor("out", [NL, C, H, W], f32, kind="ExternalOutput")

    groups = [list(range(N_CORES))]

    def interior(tile_ap, base, nrows):
        """[128, nrows, 56] strided view (row stride PW) starting at `base`."""
        v = tile_ap[:, base:base + nrows * PW].rearrange(
            "p (r c) -> p r c", c=PW)
        return v[:, :, 0:W]

    with tile.TileContext(nc) as tc:
        import contextlib
        with contextlib.ExitStack() as ctx:
            const = ctx.enter_context(tc.tile_pool(name="const", bufs=1))
            psum = ctx.enter_context(tc.tile_pool(name="psum", bufs=8, space="PSUM"))
            xst = ctx.enter_context(tc.tile_pool(name="xst", bufs=2))
            otp = ctx.enter_context(tc.tile_pool(name="otp", bufs=2))
            sqp = ctx.enter_context(tc.tile_pool(name="sqp", bufs=2))

            # persistent per-image planes
            x_pad = [[const.tile([128, PLANE], bf16, tag=f"xp{j}_{n}", name=f"xp{j}_{n}")
                      for n in range(NL)] for j in range(2)]
            h1_pad = [[const.tile([128, PLANE], bf16, tag=f"h1{j}_{n}", name=f"h1{j}_{n}")
                       for n in range(NL)] for j in range(2)]
            h2 = [[const.tile([128, HW], bf16, tag=f"h2{j}_{n}", name=f"h2{j}_{n}")
                   for n in range(NL)] for j in range(2)]

            wt_sb = [const.tile([128, 36 * 128], bf16, tag=f"wt{c}", name=f"wt{c}")
                     for c in range(2)]
            aff_sb = const.tile([128, 8], f32, tag="aff", name="aff")
            eye_sb = const.tile([128, 128], bf16, tag="eye", name="eye")

            rsem = [nc.alloc_semaphore(f"rst{i}") for i in range(4)]
            lsem = nc.alloc_semaphore("lst")
            _gp_prev = [None]
            deferred_waits = []

            def gp_order(bi):
                if _gp_prev[0] is not None:
                    bass_mod._add_dep_helper(bi.ins, _gp_prev[0].ins,
                                             sync=False,
                                             reason="stats-exchange order")
                _gp_prev[0] = bi
                return bi

            nc._bir_kernel_barrier_sem_replica_groups.extend(
                set(g) for g in groups)

            def defer_wait(bi, sem, val):
                bi._wait_ge(sem, 0)
                deferred_waits.append((bi, sem, val))
                return bi

            for i, s in enumerate(rsem + [lsem]):
                cl = gp_order(nc.gpsimd.sem_clear(s))
                if i == 0:
                    defer_wait(cl, nc._bir_kernel_barrier_sem,
                               nc.bir_kernel_barrier_sem_inc)

            # stats accumulators: one column per (image, chunk)
            acc = {(b, s, j): const.tile([128, NL * NCHUNK], f32,
                                         tag=f"acc{b}{s}{j}", name=f"acc{b}{s}{j}")
                   for b in (1, 2) for s in ("s", "q") for j in range(2)}

            rv = [const.tile([128, 16], f32, tag=f"rv{ex}", name=f"rv{ex}")
                  for ex in range(4)]
            packed = [const.tile([128, 2], f32, tag=f"pk{ex}", name=f"pk{ex}")
                      for ex in range(4)]

            def pad_memset(t):
                nc.vector.memset(t[:, 0:STRIP0], 0.0)
                pairs = t[:, 2 * PW - 1:2 * PW - 1 + 56 * PW].rearrange(
                    "p (r c) -> p r c", c=PW)[:, :, 0:2]
                nc.vector.memset(pairs, 0.0)
                nc.vector.memset(t[:, STRIP0 + 56 * PW:PLANE], 0.0)

            # ---- head: image-0 x DMAs issue first (Sync engine issues one
            # dma_start per ~0.75us, so order = priority), weights after. ----
            def x_load(n, j):
                for rh in range(2):
                    r0 = rh * HALF_ROWS
                    xs = xst.tile([128, HALF_ELEMS], bf16, tag="xs", name="xs")
                    nc.sync.dma_start(
                        xs[:], x_d[n, j * 128:(j + 1) * 128, r0:r0 + HALF_ROWS, :])
                    dst = interior(x_pad[j][n], (r0 + 1) * PW + 1, HALF_ROWS)
                    nc.vector.tensor_copy(
                        dst, xs[:, :].rearrange("p (r c) -> p r c", c=W))

            x_load(0, 0)
            x_load(0, 1)
            nc.sync.dma_start(wt_sb[0][:, 0:18 * 128], wt_d[:, 0:18 * 128])
            nc.sync.dma_start(wt_sb[0][:, 18 * 128:36 * 128],
                              wt_d[:, 18 * 128:36 * 128])
            pad_memset(x_pad[0][0])
            pad_memset(x_pad[1][0])
            for n in range(1, NL):
                for j in range(2):
                    x_load(n, j)
                pad_memset(x_pad[0][n])
                pad_memset(x_pad[1][n])
            nc.sync.dma_start(wt_sb[1][:], wt_d[:, 36 * 128:72 * 128])
            nc.sync.dma_start(aff_sb[:], aff_d[:])
            nc.sync.dma_start(eye_sb[:], eye_d[:])

            # ---- stats-exchange pieces ----
            def stats_send(bn_i, j, ex):
                nc.vector.tensor_reduce(
                    packed[ex][:, 0:1], acc[(bn_i, "s", j)][:], axis=AX.X, op=ALU.add)
                nc.vector.tensor_reduce(
                    packed[ex][:, 1:2], acc[(bn_i, "q", j)][:], axis=AX.X, op=ALU.add)
                cp = nc.vector.tensor_copy(rv[ex][:, 0:2], packed[ex][:])
                for d in range(1, 8):
                    q = (d - 1) % 4
                    rd = [None] * 8
                    rd[d] = (0, d)
                    gp_order(nc.gpsimd.remote_dma_broadcast(
                        rv[ex][:, 2 * d:2 * d + 2], packed[ex][:],
                        remote_sem=rsem[ex], local_sem=lsem, rdests=rd,
                        queue_num=q))
                for q in range(4):
                    gp_order(nc.gpsimd.trigger_dma(count=None, queue_num=q))
                return cp

            def stats_recv(bn_i, j, ex, after_bis):
                sfx = f"{bn_i}{j}"
                gl = const.tile([128, 2], f32, tag=f"gl{sfx}", name=f"gl{sfx}")
                red = nc.vector.tensor_reduce(
                    gl[:], rv[ex][:, 0:16].rearrange("p (s c) -> p c s", c=2),
                    axis=AX.X, op=ALU.add)
                defer_wait(red, rsem[ex], 14)
                for ab in after_bis:
                    bass_mod._add_dep_helper(red.ins, ab.ins, sync=True,
                                             reason="recv ordering")
                return gl

            def bn_affine_finish_fast(bn_i, j, gl, g_col, b_col, act_pin):
                """ACT-sqrt + DVE-reciprocal affine: ~1.5us instead of the
                ~3.8us DVE bit-trick chain.  Only legal post-conv (the sqrt
                sits in the ACT stream; act_pin keeps the scheduler from
                hoisting it into the conv copy stream)."""
                sfx = f"f{bn_i}{j}"
                mean = const.tile([128, 1], f32, tag=f"mean{sfx}", name=f"mean{sfx}")
                nc.vector.tensor_scalar_mul(mean[:], gl[:, 0:1], 1.0 / COUNT)
                var = const.tile([128, 1], f32, tag=f"var{sfx}", name=f"var{sfx}")
                nc.vector.tensor_tensor(var[:], mean[:], mean[:], ALU.mult)
                nc.vector.scalar_tensor_tensor(
                    var[:], gl[:, 1:2], 1.0 / COUNT, var[:],
                    ALU.mult, ALU.subtract)
                nc.vector.tensor_scalar_add(var[:], var[:], EPS)
                sd = const.tile([128, 1], f32, tag=f"sd{sfx}", name=f"sd{sfx}")
                sq = nc.scalar.sqrt(sd[:], var[:])
                bass_mod._add_dep_helper(sq.ins, act_pin.ins, sync=True,
                                         reason="affine sqrt after conv ACT")
                y = const.tile([128, 1], f32, tag=f"y{sfx}", name=f"y{sfx}")
                nc.vector.reciprocal(y[:], sd[:])
                sc = const.tile([128, 1], f32, tag=f"sc{sfx}", name=f"sc{sfx}")
                nc.vector.tensor_tensor(sc[:], aff_sb[:, g_col + j:g_col + j + 1],
                                        y[:], ALU.mult)
                bi = const.tile([128, 1], f32, tag=f"bi{sfx}", name=f"bi{sfx}")
                nc.vector.tensor_tensor(bi[:], mean[:], sc[:], ALU.mult)
                nc.vector.tensor_tensor(bi[:], aff_sb[:, b_col + j:b_col + j + 1],
                                        bi[:], ALU.subtract)
                return sc, bi

            def bn_affine_finish(bn_i, j, gl, g_col, b_col):
                sfx = f"{bn_i}{j}"
                mean = const.tile([128, 1], f32, tag=f"mean{sfx}", name=f"mean{sfx}")
                nc.vector.tensor_scalar_mul(mean[:], gl[:, 0:1], 1.0 / COUNT)
                var = const.tile([128, 1], f32, tag=f"var{sfx}", name=f"var{sfx}")
                nc.vector.tensor_tensor(var[:], mean[:], mean[:], ALU.mult)
                nc.vector.scalar_tensor_tensor(
                    var[:], gl[:, 1:2], 1.0 / COUNT, var[:],
                    ALU.mult, ALU.subtract)
                nc.vector.tensor_scalar_add(var[:], var[:], EPS)
                y = const.tile([128, 1], f32, tag=f"y{sfx}", name=f"y{sfx}")
                vh = const.tile([128, 1], f32, tag=f"vh{sfx}", name=f"vh{sfx}")
                tmp = const.tile([128, 1], f32, tag=f"tm{sfx}", name=f"tm{sfx}")
                iv = var[:].bitcast(mybir.dt.int32)
                yi = y[:].bitcast(mybir.dt.int32)
                nc.vector.tensor_scalar(yi, iv, 1, None, ALU.arith_shift_right)
                nc.vector.tensor_scalar(yi, yi, -1, None, ALU.bitwise_xor)
                nc.vector.tensor_scalar(yi, yi, 0x5f3759df + 1, None, ALU.add)
                nc.vector.tensor_scalar_mul(vh[:], var[:], 0.5)
                for _ in range(2):
                    nc.vector.tensor_tensor(tmp[:], y[:], y[:], ALU.mult)
                    nc.vector.tensor_tensor(tmp[:], tmp[:], vh[:], ALU.mult)
                    nc.vector.tensor_scalar(tmp[:], tmp[:], -1.0, 1.5,
                                            ALU.mult, ALU.add)
                    nc.vector.tensor_tensor(y[:], y[:], tmp[:], ALU.mult)
                sc = const.tile([128, 1], f32, tag=f"sc{sfx}", name=f"sc{sfx}")
                nc.vector.tensor_tensor(sc[:], aff_sb[:, g_col + j:g_col + j + 1],
                                        y[:], ALU.mult)
                bi = const.tile([128, 1], f32, tag=f"bi{sfx}", name=f"bi{sfx}")
                nc.vector.tensor_tensor(bi[:], mean[:], sc[:], ALU.mult)
                nc.vector.tensor_tensor(bi[:], aff_sb[:, b_col + j:b_col + j + 1],
                                        bi[:], ALU.subtract)
                return sc, bi

            def apply_bn1_dve(j, s, b):
                """relu(s*h1+b) in place on DVE (never blocks the ACT/PSUM
                pipeline; runs in the recv/affine shadow)."""
                for n in range(NL):
                    v = interior(h1_pad[j][n], STRIP0, H)
                    nc.vector.tensor_scalar(v, v, s[:], b[:], ALU.mult, ALU.add)
                    nc.vector.tensor_scalar(v, v, 0.0, None, ALU.max)

            # ---- conv pass emitters ----
            def conv1(j):
                last = None
                for n in range(NL):
                    for k in range(NCHUNK):
                        pt = psum.tile([128, 8 * W], f32, tag="ps", name="ps")
                        idx = 0
                        for ci in range(2):
                            for ky in range(3):
                                for kx in range(3):
                                    t = _t_index(ky, kx, ci, j)
                                    dq = (ky - 1) * PW + (kx - 1)
                                    off = STRIP0 + CHUNK * k + dq
                                    rhs = x_pad[ci][n][
                                        :, off:off + CHUNK].rearrange(
                                        "p (r c) -> p r c", c=PW)[:, :, 0:W]
                                    nc.tensor.matmul(
                                        pt[:], wt_sb[0][:, t * 128:(t + 1) * 128],
                                        rhs, start=(idx == 0), stop=(idx == 17))
                                    idx += 1
                        src_int = pt[:, 0:8 * W].rearrange("p (r c) -> p r c", c=W)
                        dst_int = interior(h1_pad[j][n], (1 + 8 * k) * PW + 1, 8)
                        col = n * NCHUNK + k
                        nc.scalar.activation(
                            dst_int, src_int, AF.Copy,
                            accum_out=acc[(1, "s", j)][:, col:col + 1])
                        sq = sqp.tile([128, 8 * W], f32, tag="sq", name="sq")
                        last = nc.scalar.activation(
                            sq[:, :].rearrange("p (r c) -> p r c", c=W),
                            dst_int, AF.Square,
                            accum_out=acc[(1, "q", j)][:, col:col + 1])
                return last

            def conv2_passA(j):
                """ci-half-0 taps only -> h2[j] partial (bf16, no stats)."""
                for n in range(NL):
                    for k in range(NCHUNK):
                        pt = psum.tile([128, 8 * W], f32, tag="ps", name="ps")
                        for ti, (ky, kx) in enumerate(
                                (ky, kx) for ky in range(3) for kx in range(3)):
                            t = 36 + _t_index(ky, kx, 0, j)
                            dq = (ky - 1) * PW + (kx - 1)
                            off = STRIP0 + CHUNK * k + dq
                            rhs = h1_pad[0][n][:, off:off + CHUNK].rearrange(
                                "p (r c) -> p r c", c=PW)[:, :, 0:W]
                            nc.tensor.matmul(
                                pt[:], wt_sb[1][:, (t - 36) * 128:(t - 35) * 128],
                                rhs, start=(ti == 0), stop=(ti == 8))
                        dst = h2[j][n][:, 8 * k * W:(8 * k + 8) * W].rearrange(
                            "p (r c) -> p r c", c=W)
                        nc.scalar.activation(
                            dst, pt[:, 0:8 * W].rearrange("p (r c) -> p r c", c=W),
                            AF.Copy)

            def conv2_passB(j):
                """identity-reload of the partial + ci-half-1 taps + stats."""
                last = None
                for n in range(NL):
                    for k in range(NCHUNK):
                        pt = psum.tile([128, 8 * W], f32, tag="ps", name="ps")
                        part = h2[j][n][:, 8 * k * W:(8 * k + 8) * W].rearrange(
                            "p (r c) -> p r c", c=W)
                        nc.tensor.matmul(pt[:], eye_sb[:], part,
                                         start=True, stop=False)
                        for ti, (ky, kx) in enumerate(
                                (ky, kx) for ky in range(3) for kx in range(3)):
                            t = 36 + _t_index(ky, kx, 1, j)
                            dq = (ky - 1) * PW + (kx - 1)
                            off = STRIP0 + CHUNK * k + dq
                            rhs = h1_pad[1][n][:, off:off + CHUNK].rearrange(
                                "p (r c) -> p r c", c=PW)[:, :, 0:W]
                            nc.tensor.matmul(
                                pt[:], wt_sb[1][:, (t - 36) * 128:(t - 35) * 128],
                                rhs, start=False, stop=(ti == 8))
                        src_int = pt[:, 0:8 * W].rearrange("p (r c) -> p r c", c=W)
                        dst = h2[j][n][:, 8 * k * W:(8 * k + 8) * W].rearrange(
                            "p (r c) -> p r c", c=W)
                        col = n * NCHUNK + k
                        nc.scalar.activation(
                            dst, src_int, AF.Copy,
                            accum_out=acc[(2, "s", j)][:, col:col + 1])
                        sq = sqp.tile([128, 8 * W], f32, tag="sq", name="sq")
                        last = nc.scalar.activation(
                            sq[:, :].rearrange("p (r c) -> p r c", c=W), dst,
                            AF.Square,
                            accum_out=acc[(2, "q", j)][:, col:col + 1])
                return last

            # ---- tail: out = relu(s2*h2 + b2 + x), stream to DRAM. ----
            def tail(j, s2, b2, mode, pin_to):
                last_dve = None
                for m, (n, rh) in enumerate((n, rh) for n in range(NL)
                                            for rh in range(2)):
                    r0 = rh * HALF_ROWS
                    xv = interior(x_pad[j][n], (r0 + 1) * PW + 1, HALF_ROWS)
                    h2v = h2[j][n][:, r0 * W:r0 * W + HALF_ELEMS].rearrange(
                        "p (r c) -> p r c", c=W)
                    pool = otp if m % 2 == 0 else xst
                    ot = pool.tile([128, HALF_ELEMS], f32,
                                   tag="ot" if m % 2 == 0 else "xs", name="ot")
                    otv = ot[:, :].rearrange("p (r c) -> p r c", c=W)
                    if mode == "dve":
                        stt = nc.vector.scalar_tensor_tensor(
                            otv, h2v, s2[:], xv, ALU.mult, ALU.add)
                        if m == 0 and pin_to is not None:
                            bass_mod._add_dep_helper(stt.ins, pin_to.ins,
                                                     sync=True,
                                                     reason="tail order")
                        last_dve = nc.vector.tensor_scalar(
                            ot[:], ot[:], b2[:], 0.0, ALU.add, ALU.max)
                    else:
                        # bf16 in-place add-residual on the DVE (2 elem/cycle:
                        # ~0.95us vs 1.85us f32); the ACT relu converts
                        # bf16 -> f32 on the way to the output tile
                        stt = nc.vector.scalar_tensor_tensor(
                            h2v, h2v, s2[:], xv, ALU.mult, ALU.add)
                        if m == 0 and pin_to is not None:
                            bass_mod._add_dep_helper(stt.ins, pin_to.ins,
                                                     sync=True,
                                                     reason="tail order")
                        last_dve = stt
                        rl = nc.scalar.activation(otv, h2v, AF.Relu,
                                                  bias=b2[:], scale=1.0)
                        if pin_to is not None:
                            bass_mod._add_dep_helper(rl.ins, pin_to.ins,
                                                     sync=True,
                                                     reason="tail relu pin")
                    nc.sync.dma_start(
                        out_d[n, j * 128:(j + 1) * 128, r0:r0 + HALF_ROWS, :],
                        ot[:])
                return last_dve

            # ================= phase schedule =================
            conv1(0)                                         # conv1 half 0
            cp10 = stats_send(1, 0, 0)
            # h1 pad zeroing on DVE, emitted BEFORE the blocking recv so
            # conv1-half-1's ACT copies can never queue behind it
            for jj in range(2):
                for n in range(NL):
                    pad_memset(h1_pad[jj][n])
            gl10 = stats_recv(1, 0, 0, [cp10])               # blocks idle DVE
            s1_0, b1_0 = bn_affine_finish(1, 0, gl10, 0, 2)
            apply_bn1_dve(0, s1_0, b1_0)                     # in recv shadow
            c1_last = conv1(1)                               # conv1 half 1
            cp11 = stats_send(1, 1, 1)
            gl11 = stats_recv(1, 1, 1, [cp11])
            s1_1, b1_1 = bn_affine_finish(1, 1, gl11, 0, 2)
            apply_bn1_dve(1, s1_1, b1_1)
            conv2_passA(0)                                   # needs apply j0 only
            conv2_passA(1)
            c2a_last = conv2_passB(0)                        # needs apply j1
            cp20 = stats_send(2, 0, 2)
            gl20 = stats_recv(2, 0, 2, [cp20])
            s2_0, b2_0 = bn_affine_finish(2, 0, gl20, 4, 6)
            # tail half 0: DVE-only, overlaps pass-B half 1
            t0_last = tail(0, s2_0, b2_0, mode="dve", pin_to=cp20)
            c2_last = conv2_passB(1)                         # conv2 half 1
            cp21 = stats_send(2, 1, 3)
            gl21 = stats_recv(2, 1, 3, [cp21, t0_last])
            s2_1, b2_1 = bn_affine_finish_fast(2, 1, gl21, 4, 6, c2_last)
            tail(1, s2_1, b2_1, mode="mixed", pin_to=c2_last)

    for bi, sem, val in deferred_waits:
        patched = False
        for w in bi.ins.sync_info.on_wait:
            if w.id == sem.num and w.wait_value == 0:
                w.wait_value = val
                patched = True
                break
        assert patched, f"deferred wait not found on {bi.ins.name}"

    nc.compile()
    return nc


def kernel(x, W1, W2, gamma1, beta1, gamma2, beta2, mask1, mask2,
           _trace=False, _trace_kwargs=None):
    from concourse.bass_utils import run_bass_kernel_spmd

    if "nc" not in _cache:
        _cache["nc"] = _build()
    nc = _cache["nc"]

    wt = _pack_weights(np.asarray(W1, np.float32), np.asarray(W2, np.float32),
                       np.asarray(mask1, np.float32), np.asarray(mask2, np.float32))
    aff = _pack_aff(gamma1, beta1, gamma2, beta2)
    eye = np.eye(128, dtype=_BF16)
    x = np.ascontiguousarray(np.asarray(x, np.float32).astype(_BF16))

    in_maps = [{"x": x[i * NL:(i + 1) * NL], "wt": wt, "aff": aff, "eye": eye}
               for i in range(N_CORES)]
    kw = {}
    if _trace:
        kw = dict(trace=True, **(_trace_kwargs or {}))
    res = run_bass_kernel_spmd(nc, in_maps, core_ids=list(range(N_CORES)), **kw)
    out = np.concatenate([res.results[i]["out"] for i in range(N_CORES)], axis=0)
    _cache["last_results"] = res
    return out
